# revision 19
# baseline (speedup 1.0000x reference)
"""KACN (Chebyshev MLP) Trainium2 kernel.

Math: reference layer is  einsum('bid,iod->bo', cos(d*arccos(tanh x)), C)
which is exactly sum_d T_d(tanh x) @ C[:,:,d]  (Chebyshev polynomials).
With t = tanh(x):
  T_0 = 1, T_1 = t, T_2 = 2t^2 - 1, T_3 = 4t^3 - 3t
=> layer(x) = bias + t @ A1 + t^2 @ A2 + t^3 @ A3
   A1 = C1 - 3*C3, A2 = 2*C2, A3 = 4*C3, bias_o = sum_i (C0 - C2)[i,o]

Per-core plan (batch shard 2048 of 16384, weights replicated):
  - host pre-transposes x to feature-major bf16, so activations are computed
    directly in the matmul contraction layout (K on partitions); no on-device
    transposes anywhere.
  - layer1 in fp8 e4m3 + DoubleRow: weights host-scaled by 2^12 (clipped to
    +-224; TRN e4m3 saturates at 240), 9 K-pairs of 256 rows per of-block
    plus one packed 48-row bf16 tail block. ACT evacuates PSUM with fused
    tanh + bias + 2^-12 descale -> u^T bf16 (already the layer-2 layout).
  - layer2 stays bf16 (u ~ 1e-2 would sit in fp8 subnormals): 24 K-blocks
    accumulated into a pinned (10, batch-half) PSUM region, lagged one
    of-block behind layer 1 so the PE never waits on the activation chain.
  - batch-half pipelining: t/t^2/t^3 production of one 1024-col half runs on
    ACT/DVE while the PE processes the other half; DMA issue order follows
    the consumption critical path; warm-up matmuls hold the HAM clock gate
    at 2.4 GHz through the DMA-bound prologue.
  - output returned as y^T (10, 2048) f32; host transposes + concats shards.
"""

import numpy as np
import ml_dtypes

DEGREE = 3
I0, H, O = 784, 1024, 10
B = 16384
N_CORES = 8
BS = B // N_CORES  # 2048 batch rows per core

FB_FULL = I0 // 128          # 6 full feature blocks of layer-1 input
FB_TAIL = I0 - FB_FULL * 128  # 16
K1_BLOCKS = 3 * FB_FULL + 1   # 18 full + 1 packed tail (3*16=48 rows)
OF1 = H // 128                # 8 output-feature blocks of layer 1
K2_BLOCKS = 3 * OF1           # 24
NBC = BS // 512               # moving-operand chunks of 512

_cache = {}


def _build_program():
    import concourse.bass as bass
    import concourse.mybir as mybir
    import concourse.tile as tile
    from concourse import bacc

    f32 = mybir.dt.float32
    bf16 = mybir.dt.bfloat16
    f8 = mybir.dt.float8e4
    AF = mybir.ActivationFunctionType
    DR = mybir.MatmulPerfMode.DoubleRow

    nc = bacc.Bacc("TRN2", target_bir_lowering=False, debug=False)

    xt_d = nc.dram_tensor("xt", (I0, BS), bf16, kind="ExternalInput").ap()
    w1_d = nc.dram_tensor("w1", (18 * 128, H), f8, kind="ExternalInput").ap()
    w1t_d = nc.dram_tensor("w1t", (48, H), bf16, kind="ExternalInput").ap()
    b1_d = nc.dram_tensor("b1", (128, OF1), f32, kind="ExternalInput").ap()
    w2_d = nc.dram_tensor("w2", (3 * H, O), bf16, kind="ExternalInput").ap()
    b2_d = nc.dram_tensor("b2", (O, 1), f32, kind="ExternalInput").ap()
    yt_d = nc.dram_tensor("yt", (O, BS), f32, kind="ExternalOutput").ap()

    with tile.TileContext(nc) as tc:
        with (
            tc.tile_pool(name="wpool", bufs=1) as wpool,
            tc.tile_pool(name="xpool", bufs=3) as xpool,
            tc.tile_pool(name="tpool", bufs=1) as tpool,
            tc.tile_pool(name="upool", bufs=3) as upool,
            tc.tile_pool(name="ypool", bufs=1) as ypool,
            tc.tile_pool(name="psum1", bufs=3, space="PSUM") as psum1,
            tc.tile_pool(name="psum2", bufs=1, space="PSUM") as psum2,
        ):
            HB = BS // 2  # 1024-column batch halves

            # ---- layer-1 weights (fp8, 18 K-blocks) + bf16 tail ----
            w1_sb = wpool.tile([128, 18, H], f8, tag="w1")
            w1t_sb = wpool.tile([48, H], bf16, tag="w1t")

            # ---- activation storage ----
            t_sb = tpool.tile([128, FB_FULL, BS], f8, tag="t1")
            t2_sb = tpool.tile([128, FB_FULL, BS], f8, tag="t2")
            t3_sb = tpool.tile([128, FB_FULL, BS], f8, tag="t3")
            tail_sb = tpool.tile([48, BS], bf16, tag="tail")

            # PE warm-up: serial tiny matmuls keep the HAM activity window
            # busy through the DMA-bound prologue so real matmuls start at
            # 2.4 GHz instead of the cold 1.2 GHz.
            wz = xpool.tile([128, 128], f8, tag="wz")
            nc.gpsimd.memset(wz[:, :], 0.0)
            pwarm = psum1.tile([128, 64], f32, tag="p1", name="pwarm")
            for i in range(60):
                nc.tensor.matmul(
                    pwarm[:, :], wz[:, :], wz[:, 0:64], start=True, stop=True
                )

            # DMA issue order tracks the consumption critical path: the
            # activation pipeline needs xt first; w1 pair-group e is not
            # needed until the of-loop reaches it.
            xtl = xpool.tile([16, BS], bf16, tag="xtl")
            nc.sync.dma_start(out=xtl[:, :], in_=xt_d[FB_FULL * 128 :, :])
            xt_tiles = []
            for fb in range(FB_FULL):
                xt_t = xpool.tile([128, BS], bf16, tag="xt", name=f"xt{fb}", bufs=6)
                nc.sync.dma_start(
                    out=xt_t[:, :], in_=xt_d[fb * 128 : (fb + 1) * 128, :]
                )
                xt_tiles.append(xt_t)
                if fb % 2 == 1:
                    e = fb // 2
                    for poly in range(3):
                        for i in range(2):
                            k = poly * 6 + 2 * e + i
                            nc.sync.dma_start(
                                out=w1_sb[:, k, :],
                                in_=w1_d[k * 128 : (k + 1) * 128, :],
                            )
            nc.sync.dma_start(out=w1t_sb[:, :], in_=w1t_d[:, :])

            # tail chain (tiny, produced once up front)
            ttl = xpool.tile([16, BS], bf16, tag="ttl")
            t2tl = xpool.tile([16, BS], bf16, tag="t2tl")
            t3tl = xpool.tile([16, BS], bf16, tag="t3tl")
            nc.scalar.activation(ttl[:, :], xtl[:, :], AF.Tanh)
            nc.vector.tensor_mul(t2tl[:, :], ttl[:, :], ttl[:, :])
            nc.vector.tensor_mul(t3tl[:, :], t2tl[:, :], ttl[:, :])
            w2_sb = wpool.tile([128, K2_BLOCKS, O], bf16, tag="w2")
            nc.sync.dma_start(
                out=w2_sb[:, :, :],
                in_=w2_d.rearrange("(k p) n -> p k n", p=128),
            )
            b1_sb = wpool.tile([128, OF1], f32, tag="b1")
            nc.sync.dma_start(out=b1_sb[:, :], in_=b1_d[:, :])
            b2_sb = wpool.tile([O, 1], f32, tag="b2")
            nc.sync.dma_start(out=b2_sb[:, :], in_=b2_d[:, :])
            nc.sync.dma_start(out=tail_sb[0:16, :], in_=ttl[:, :])
            nc.sync.dma_start(out=tail_sb[16:32, :], in_=t2tl[:, :])
            nc.sync.dma_start(out=tail_sb[32:48, :], in_=t3tl[:, :])

            def produce(half):
                """t/t^2/t^3 (fp8) for one 1024-col batch half; t^2 split
                across ACT/DVE to balance engine load."""
                hl = slice(half * HB, (half + 1) * HB)
                for fb in range(FB_FULL):
                    nc.scalar.activation(
                        t_sb[:, fb, hl], xt_tiles[fb][:, hl], AF.Tanh
                    )
                    if fb % 2 == 0 and half == 1:
                        nc.scalar.activation(
                            t2_sb[:, fb, hl], t_sb[:, fb, hl], AF.Square
                        )
                    else:
                        nc.vector.tensor_mul(
                            t2_sb[:, fb, hl], t_sb[:, fb, hl], t_sb[:, fb, hl]
                        )
                    nc.vector.tensor_mul(
                        t3_sb[:, fb, hl], t2_sb[:, fb, hl], t_sb[:, fb, hl]
                    )

            def run_half(half):
                hoff = half * HB
                yp = psum2.tile([O, HB], f32, tag="yp", name=f"yp{half}")
                prev = None
                for of in range(OF1 + 1):
                    if of < OF1:
                        pp = psum1.tile(
                            [128, HB], f32, tag="p1", name=f"p1_{half}_{of}"
                        )
                        ofs = slice(of * 128, (of + 1) * 128)
                        for j in range(9):
                            e, poly = divmod(j, 3)
                            kk = poly * 6 + 2 * e
                            lhsT = w1_sb[:, kk : kk + 2, ofs]
                            rhs_t = (t_sb, t2_sb, t3_sb)[poly]
                            for sub in range(2):
                                sl = slice(hoff + sub * 512, hoff + (sub + 1) * 512)
                                nc.tensor.matmul(
                                    pp[:, sub * 512 : (sub + 1) * 512],
                                    lhsT,
                                    rhs_t[:, 2 * e : 2 * e + 2, sl],
                                    start=(j == 0),
                                    stop=False,
                                    perf_mode=DR,
                                )
                        for sub in range(2):
                            sl = slice(hoff + sub * 512, hoff + (sub + 1) * 512)
                            nc.tensor.matmul(
                                pp[:, sub * 512 : (sub + 1) * 512],
                                w1t_sb[:, ofs],
                                tail_sb[:, sl],
                                start=False,
                                stop=True,
                            )
                        u = upool.tile([128, HB], bf16, tag="u", name=f"u_{half}_{of}")
                        u2 = upool.tile([128, HB], bf16, tag="u2", name=f"u2_{half}_{of}")
                        u3 = upool.tile([128, HB], bf16, tag="u3", name=f"u3_{half}_{of}")
                        for sub in range(2):
                            ps = slice(sub * 512, (sub + 1) * 512)
                            nc.scalar.activation(
                                u[:, ps], pp[:, ps], AF.Tanh,
                                bias=b1_sb[:, of : of + 1], scale=float(2.0 ** -12),
                            )
                            if of % 2 == 0 or half == 0:
                                nc.scalar.activation(
                                    u2[:, ps], u[:, ps], AF.Square
                                )
                            else:
                                nc.vector.tensor_mul(u2[:, ps], u[:, ps], u[:, ps])
                            nc.vector.tensor_mul(u3[:, ps], u2[:, ps], u[:, ps])
                        cur = (of, [u, u2, u3])
                    else:
                        cur = None

                    if prev is not None:
                        pof, polys = prev
                        for sub in range(2):
                            for poly in range(3):
                                k2 = pof * 3 + poly
                                nc.tensor.matmul(
                                    yp[:, sub * 512 : (sub + 1) * 512],
                                    w2_sb[:, k2, :],
                                    polys[poly][:, sub * 512 : (sub + 1) * 512],
                                    start=(k2 == 0),
                                    stop=(k2 == K2_BLOCKS - 1),
                                )
                    prev = cur

                y_sb = ypool.tile([O, HB], f32, tag="y", name=f"y{half}")
                for sub in range(2):
                    ps = slice(sub * 512, (sub + 1) * 512)
                    nc.scalar.activation(
                        y_sb[:, ps], yp[:, ps], AF.Identity, bias=b2_sb[:, :],
                    )
                    nc.sync.dma_start(
                        out=yt_d[:, hoff + sub * 512 : hoff + (sub + 1) * 512],
                        in_=y_sb[:, ps],
                    )

            produce(0)
            run_half(0)
            produce(1)
            run_half(1)

    nc.compile()
    return nc


def _prep(x, coeffs0, coeffs1):
    bf = ml_dtypes.bfloat16
    c0 = np.asarray(coeffs0, np.float32)
    c1 = np.asarray(coeffs1, np.float32)

    def combine(c):
        A1 = c[:, :, 1] - 3.0 * c[:, :, 3]
        A2 = 2.0 * c[:, :, 2]
        A3 = 4.0 * c[:, :, 3]
        bias = (c[:, :, 0] - c[:, :, 2]).sum(axis=0)
        return A1, A2, A3, bias

    A1, A2, A3, bias0 = combine(c0)
    B1, B2, B3, bias1 = combine(c1)

    nfull = FB_FULL * 128
    f8 = ml_dtypes.float8_e4m3
    w1 = np.concatenate([A1[:nfull], A2[:nfull], A3[:nfull]], axis=0)
    w1 = np.clip(w1 * 4096.0, -224.0, 224.0).astype(f8)
    w1t = np.concatenate([A1[nfull:], A2[nfull:], A3[nfull:]], axis=0).astype(bf)
    # layer-2 K order: for of-block: B1,B2,B3 rows of that block
    w2 = np.concatenate(
        [Bp[of * 128 : (of + 1) * 128] for of in range(OF1) for Bp in (B1, B2, B3)],
        axis=0,
    ).astype(bf)
    b1 = np.ascontiguousarray(bias0.reshape(OF1, 128).T.astype(np.float32))
    b2 = bias1.reshape(O, 1).astype(np.float32)

    xt = np.ascontiguousarray(np.asarray(x, np.float32).T.astype(bf))  # (784, B)
    return xt, w1, w1t, b1, w2, b2


def _install_profile_shim():
    """Register the NTFF profile hook (missing antenv.axon_hooks in this
    image) and neuter the S3 artifact upload. Test-time only."""
    import sys
    import types
    import ctypes
    import contextlib

    if "antenv.axon_hooks" in sys.modules:
        return
    so_path = "/opt/axon/libaxon_pjrt.so"
    lib = ctypes.CDLL(so_path)
    if not hasattr(lib, "axon_start_nrt_profile"):
        return
    lib.axon_start_nrt_profile.argtypes = [
        ctypes.POINTER(ctypes.c_int64),
        ctypes.c_size_t,
    ]
    lib.axon_start_nrt_profile.restype = ctypes.c_int64
    lib.axon_stop_nrt_profile.argtypes = [ctypes.c_char_p]
    lib.axon_stop_nrt_profile.restype = ctypes.c_int64

    @contextlib.contextmanager
    def _hook(output_dir, device_ids):
        import jax

        jax.devices()
        if device_ids:
            ids = (ctypes.c_int64 * len(device_ids))(*device_ids)
            rc = lib.axon_start_nrt_profile(ids, len(device_ids))
        else:
            rc = lib.axon_start_nrt_profile(None, 0)
        if rc != 0:
            raise RuntimeError(f"axon_start_nrt_profile rc={rc}")
        try:
            yield
        finally:
            n = lib.axon_stop_nrt_profile(str(output_dir).encode())
            print(f"profile: {n} file(s) written to {output_dir}")

    mod = types.ModuleType("antenv.axon_hooks")
    mod.get_axon_ntff_profile_hook = lambda: _hook
    mod.set_axon_ntff_profile_hook = lambda h: None
    sys.modules["antenv.axon_hooks"] = mod

    import concourse.bass_utils as bu

    bu.upload_artifacts = lambda tmpdir: "local://" + str(tmpdir)


def _forward(inputs, trace=False):
    from concourse.bass_utils import run_bass_kernel_spmd

    if trace:
        _install_profile_shim()

    x = np.asarray(inputs["x"])
    xt, w1, w1t, b1, w2, b2 = _prep(x, inputs["coeffs0"], inputs["coeffs1"])

    if "nc" not in _cache:
        _cache["nc"] = _build_program()
    nc = _cache["nc"]

    in_maps = []
    for c in range(N_CORES):
        in_maps.append(
            {
                "xt": np.ascontiguousarray(xt[:, c * BS : (c + 1) * BS]),
                "w1": w1,
                "w1t": w1t,
                "b1": b1,
                "w2": w2,
                "b2": b2,
            }
        )
    res = run_bass_kernel_spmd(nc, in_maps, core_ids=list(range(N_CORES)), trace=trace)
    y = np.concatenate([r["yt"].T for r in res.results], axis=0)
    return np.ascontiguousarray(y.astype(np.float32)), res.exec_time_ns


def kernel(**inputs):
    return _forward(inputs, trace=False)[0]
